# revision 1
# baseline (speedup 1.0000x reference)
"""Bass/Tile kernel for nn_Diffeo: horizontal bilinear remap as banded matmul.

v3: fp32r main matmuls (1 cyc/row at N>=256); exact t = xn - p via a K=4
fp32r matmul whose rhs rows are a 3-way 9-bit-mantissa split of xn (each
piece is exactly representable at fp32r ingest precision, so t is exact);
band-limited W generation into persistent zero-padded weight tiles.

Per core (H-sharded, 64 rows y, all 192 b*c planes):
  out[bc, y, x_out] = sum_{x_in} imgT[y, x_in, bc] * hat(x_in - xn[y, x_out])
with hat(t) = max(0, 1-|t|):
  t[p, f]    = xn[f] - p                 (K=4 fp32r matmul, exact)
  a          = |t - 128k|                (ACT Abs, bias AP = -128k)
  wneg       = min(a - 1, 0) = -hat      (DVE tensor_scalar, output fp32r)
  psum_o    += D_k.T @ wneg              (PE, fp32r operands, fp32 accum)
  out_sb     = -psum_o                   (negate on PSUM->SBUF copy)
"""

import sys
from contextlib import ExitStack

sys.path.insert(0, "/opt/trn_rl_repo")

import numpy as np

import concourse.bass as bass
import concourse.mybir as mybir
import concourse.tile as tile
from concourse import bacc
from concourse._compat import axon_active

F32 = mybir.dt.float32
F32R = mybir.dt.float32r

H = W = 512
NPLANE = 192            # 64 batches * 3 channels
NCORES = 8
YPC = H // NCORES       # 64 rows per core
YG = 8                  # rows per input-DMA group
NG = YPC // YG          # 8 groups
KBLK = 4                # x_in blocks of 128

# matmul output windows (k=0 full width so start=True covers the PSUM bank;
# k>0 are 256 wide for the fp32r 1 cyc/row fast path); W values are only
# generated on the (narrower) band window, the rest of each weight tile is
# persistent zeros.
MS = [0, 96, 224, 256]          # matmul window starts
ML = [512, 256, 256, 256]       # matmul window lengths
GS = [0, 100, 228, 320]         # generation (band) window starts
GL = [176, 192, 192, 192]       # generation window lengths


def build_program(num_devices: int = NCORES):
    nc = bacc.Bacc(
        "TRN2",
        target_bir_lowering=False,
        debug=not axon_active(),
        num_devices=num_devices,
    )
    imgT = nc.dram_tensor("imgT", [NG, W, YG * NPLANE], F32R, kind="ExternalInput").ap()
    xn4 = nc.dram_tensor("xn4", [16, NG * YG // 4 * W], F32R, kind="ExternalInput").ap()
    c4 = nc.dram_tensor("c4", [128, 128], F32R, kind="ExternalInput").ap()
    kb = nc.dram_tensor("kb", [128, KBLK], F32, kind="ExternalInput").ap()
    zz = nc.dram_tensor("zz", [128, 1024], F32R, kind="ExternalInput").ap()
    out = nc.dram_tensor("out", [YPC, NPLANE, W], F32, kind="ExternalOutput").ap()

    with tile.TileContext(nc) as tc, ExitStack() as ctx:
        const_pool = ctx.enter_context(tc.tile_pool(name="const", bufs=1))
        dpool = ctx.enter_context(tc.tile_pool(name="dt", bufs=3))
        tpool = ctx.enter_context(tc.tile_pool(name="psum_t", bufs=2, space="PSUM"))
        opool1 = ctx.enter_context(tc.tile_pool(name="psum_o1", bufs=2, space="PSUM"))
        opool2 = ctx.enter_context(tc.tile_pool(name="psum_o2", bufs=2, space="PSUM"))
        apool = ctx.enter_context(tc.tile_pool(name="abs", bufs=3))
        spool = ctx.enter_context(tc.tile_pool(name="out_sb", bufs=3))

        NF = NG * YG // 4 * W
        xn4_sb = const_pool.tile([128, NF], F32R)
        for a in range(4):
            nc.sync.dma_start(xn4_sb[32 * a : 32 * a + 4, :], xn4[4 * a : 4 * a + 4, :])
        c4_sb = const_pool.tile([128, 128], F32R)
        nc.sync.dma_start(c4_sb[:], c4[:])
        kb_sb = const_pool.tile([128, KBLK], F32)
        nc.sync.dma_start(kb_sb[:], kb[:])
        # persistent double-width weight tiles (one half per y of a pair);
        # pad regions outside the generation windows stay zero forever
        wts = []
        for k in range(KBLK):
            wt = const_pool.tile([128, 2 * ML[k]], F32R, tag=f"w{k}")
            nc.sync.dma_start(wt[:], zz[:, 0 : 2 * ML[k]])
            wts.append(wt)

        for g in range(NG):
            dtk = []
            for k in range(KBLK):
                dte = dpool.tile([128, YG * NPLANE], F32R, tag=f"dt{k}")
                nc.sync.dma_start(dte[:], imgT[g][128 * k : 128 * k + 128, :])
                dtk.append(dte)
            for yy0 in range(0, YG, 2):
                # --- weight generation for the pair (y0, y0+1) ---
                psum_t = tpool.tile([128, 2 * W], F32)
                for h in range(2):
                    y = g * YG + yy0 + h
                    a32, f = 32 * (y % 4), y // 4
                    nc.tensor.matmul(
                        psum_t[:, h * W : (h + 1) * W],
                        lhsT=c4_sb[a32 : a32 + 4, :],
                        rhs=xn4_sb[a32 : a32 + 4, f * W : (f + 1) * W],
                        start=True,
                        stop=True,
                        tile_position=(a32, 0),
                    )
                for k in range(KBLK):
                    at = apool.tile([128, 2, GL[k]], F32, tag="abs")
                    nc.scalar.activation(
                        at[:],
                        psum_t[:].rearrange("p (h x) -> p h x", h=2)[
                            :, :, GS[k] : GS[k] + GL[k]
                        ],
                        mybir.ActivationFunctionType.Abs,
                        bias=kb_sb[:, k : k + 1],
                        scale=1.0,
                    )
                    off = GS[k] - MS[k]
                    nc.vector.tensor_scalar(
                        wts[k][:].rearrange("p (h x) -> p h x", h=2)[
                            :, :, off : off + GL[k]
                        ],
                        at[:],
                        1.0,
                        0.0,
                        op0=mybir.AluOpType.subtract,
                        op1=mybir.AluOpType.min,
                    )
                # --- matmuls + output; planes 128-191 of the two y's are
                # packed into one [64, 1024] tile and one DMA ---
                osb2 = spool.tile([64, 2 * W], F32, tag="o2")
                for h in range(2):
                    yy = yy0 + h
                    y = g * YG + yy
                    psum_o1 = opool1.tile([128, W], F32)
                    psum_o2 = opool2.tile([64, W], F32)
                    base = yy * NPLANE
                    for k in range(KBLK):
                        dslice = dtk[k][:, base : base + NPLANE]
                        orng = slice(MS[k], MS[k] + ML[k])
                        rhs = wts[k][:, h * ML[k] : (h + 1) * ML[k]]
                        nc.tensor.matmul(
                            psum_o1[:, orng],
                            lhsT=dslice[:, 0:128],
                            rhs=rhs,
                            start=(k == 0),
                            stop=(k == KBLK - 1),
                        )
                        nc.tensor.matmul(
                            psum_o2[:, orng],
                            lhsT=dslice[:, 128:192],
                            rhs=rhs,
                            start=(k == 0),
                            stop=(k == KBLK - 1),
                        )
                    osb1 = spool.tile([128, W], F32, tag="o1")
                    nc.vector.tensor_scalar_mul(osb1[:], psum_o1[:], -1.0)
                    nc.scalar.mul(osb2[:, h * W : (h + 1) * W], psum_o2[:], -1.0)
                    nc.sync.dma_start(out[y, 0:128, :], osb1[:])
                y0 = g * YG + yy0
                nc.sync.dma_start(
                    out[y0 : y0 + 2, 128:NPLANE, :].rearrange("h p x -> p h x"),
                    osb2[:].rearrange("p (h x) -> p h x", h=2),
                )

    nc.compile()
    return nc


# ---------------- host-side helpers ----------------

def host_xn(c_u: np.ndarray) -> np.ndarray:
    """float64 reproduction of the reference displacement; returns xn [H, W] f32."""
    import math

    CUT = 16
    k = np.arange(1, CUT + 1, dtype=np.float64)
    i, j = np.meshgrid(k, k, indexing="ij")
    r = np.sqrt(i * i + j * j)
    e = (r < CUT + 0.5).astype(np.float64) / r
    x = np.linspace(0.0, 1.0, W, dtype=np.float64)
    s = np.sin(np.pi * x[:, None] * k[None, :])
    u = np.einsum("ij,xi,yj->yx", c_u.astype(np.float64) * e, s, s)
    Tw = 4.0 / (math.pi**3 * CUT**2 * math.log(CUT))
    dx = math.sqrt(Tw) * u * W
    xg = np.arange(W, dtype=np.float64)
    return np.clip(xg[None, :] - dx, 0.0, W - 1.0).astype(np.float32)


def _mask9(v: np.ndarray) -> np.ndarray:
    """Truncate fp32 mantissa to 9 explicit bits (exact at fp32r ingest)."""
    return (v.view(np.uint32) & np.uint32(0xFFFFC000)).view(np.float32)


def host_prep(img: np.ndarray, c_u: np.ndarray):
    """Build per-core input maps."""
    xn = host_xn(c_u)
    planes = img.reshape(NPLANE, H, W)
    # imgT_all[core, g, x, yy, p] = planes[p, YPC*core + YG*g + yy, x]
    imgT_all = np.ascontiguousarray(
        planes.reshape(NPLANE, NCORES, NG, YG, W).transpose(1, 2, 4, 3, 0)
    )
    kbias = np.zeros((128, KBLK), np.float32)
    for k in range(KBLK):
        kbias[:, k] = -128.0 * k
    c4m = np.zeros((128, 128), np.float32)
    p = np.arange(128, dtype=np.float32)
    for a in range(4):
        c4m[32 * a + 0, :] = 1.0
        c4m[32 * a + 1, :] = 1.0
        c4m[32 * a + 2, :] = 1.0
        c4m[32 * a + 3, :] = -p
    # 3-way 9-bit split of xn: xh + xm + xl == xn exactly
    xh = _mask9(xn)
    r = (xn - xh).astype(np.float32)
    xm = _mask9(r)
    xl = (r - xm).astype(np.float32)
    in_maps = []
    NF = NG * YG // 4 * W
    zz = np.zeros((128, 1024), np.float32)
    for core in range(NCORES):
        xns = np.zeros((16, NF), np.float32)
        for y in range(YPC):
            a, f = y % 4, y // 4
            yg = core * YPC + y
            xns[4 * a + 0, f * W : (f + 1) * W] = xh[yg]
            xns[4 * a + 1, f * W : (f + 1) * W] = xm[yg]
            xns[4 * a + 2, f * W : (f + 1) * W] = xl[yg]
            xns[4 * a + 3, f * W : (f + 1) * W] = 1.0
        in_maps.append(
            {
                "imgT": imgT_all[core].reshape(NG, W, YG * NPLANE),
                "xn4": xns,
                "c4": c4m,
                "kb": kbias,
                "zz": zz,
            }
        )
    return in_maps


def host_gather(outs: list) -> np.ndarray:
    """Assemble per-core 'out' [YPC, NPLANE, W] into [64, 3, H, W]."""
    full = np.empty((64, 3, H, W), np.float32)
    for core, om in enumerate(outs):
        o = om["out"].transpose(1, 0, 2)  # [NPLANE, YPC, W]
        full[:, :, core * YPC : (core + 1) * YPC, :] = o.reshape(64, 3, YPC, W)
    return full


# ---------------- harness entry point ----------------

_NC_CACHE = {}


def kernel(img: "np.ndarray", c_u: "np.ndarray", c_v: "np.ndarray") -> "np.ndarray":
    """Full-input entry: shard across 8 NeuronCores, run, reassemble."""
    img = np.ascontiguousarray(np.asarray(img, dtype=np.float32))
    c_u = np.asarray(c_u, dtype=np.float32)
    in_maps = host_prep(img, c_u)
    if "nc" not in _NC_CACHE:
        _NC_CACHE["nc"] = build_program(num_devices=NCORES)
    from concourse.bass_utils import run_bass_kernel_spmd

    res = run_bass_kernel_spmd(
        _NC_CACHE["nc"], in_maps, core_ids=list(range(NCORES)), trace=False
    )
    return host_gather(res.results)



# revision 9
# speedup vs baseline: 1.2382x; 1.2382x over previous
"""Bass/Tile kernel for nn_Diffeo: horizontal bilinear remap as banded matmul.

v4: fp16 end-to-end (image, weights, |t| tile, output) -> half the HBM
traffic of v3 and DVE 4x / ACT 2x fast paths; uniform 192-wide stacked
weight windows with the per-block -128k shift baked into the host-side
xh piece (so one Abs covers all 4 blocks, no per-block bias); main
matmuls use split start=True pieces so no full-width zeroing pass is
needed (tile-granular WAW deps keep PE program order); output stored
plane-major in DRAM for 4KB-contiguous DMA runs, quad-packed SBUF tiles.

Per core (H-sharded, 64 rows y, all 192 b*c planes):
  t[p, (k,x)] = (xh'-128k) + xm - p      (K=3 fp16 matmul, ~2^-13 exact)
  a           = |t|                      (ACT Abs cols [0,C), DVE abs_max rest)
  wneg        = min(a - 1, 0) = -hat     (DVE tensor_scalar, fp16 4x)
  psum_o     += D_k.T @ wneg             (PE fp16, split-start windows)
  out_sb      = -psum_o                  (ACT Copy scale=-1, 2x, fp16 out)
"""

import sys
from contextlib import ExitStack

sys.path.insert(0, "/opt/trn_rl_repo")

import numpy as np

import concourse.bass as bass
import concourse.mybir as mybir
import concourse.tile as tile
from concourse import bacc
from concourse._compat import axon_active

F32 = mybir.dt.float32
F16 = mybir.dt.float16

H = W = 512
NPLANE = 192            # 64 batches * 3 channels
NCORES = 8
YPC = H // NCORES       # 64 rows per core
YG = 8                  # rows per input-DMA group
NG = YPC // YG          # 8 groups
KBLK = 4                # x_in blocks of 128
GS = [0, 107, 235, 336] # window starts (out-x) per k block
GL = 176                # uniform window length (|dx| <= 21 both ways, asserted)
SW = KBLK * GL          # stacked window width = 704

# split pieces per k: (c0, c1, out0, start) relative to the window;
# each start=False piece overlaps the preceding start=True piece's region,
# so tile-granular WAW deps preserve the required order.
PIECES = [
    [(0, 176, 0, True)],
    [(0, 69, 107, False), (69, 176, 176, True)],
    [(0, 48, 235, False), (48, 176, 283, True)],
    [(0, 75, 336, False), (75, 176, 411, True)],
]


def build_program(num_devices: int = NCORES):
    nc = bacc.Bacc(
        "TRN2",
        target_bir_lowering=False,
        debug=not axon_active(),
        num_devices=num_devices,
    )
    imgT = nc.dram_tensor("imgT", [NG, W, YG * NPLANE], F16, kind="ExternalInput").ap()
    xn3 = nc.dram_tensor("xn3", [12, (YPC // 4) * SW], F16, kind="ExternalInput").ap()
    c4 = nc.dram_tensor("c4", [128, 128], F16, kind="ExternalInput").ap()
    # planes 0-127: [plane, y, x]; planes 128-191: [y%2, plane-128, y//2, x]
    out = nc.dram_tensor("out", [128, YPC, W], F16, kind="ExternalOutput").ap()
    out2 = nc.dram_tensor("out2", [2, 64, YPC // 2, W], F16, kind="ExternalOutput").ap()

    with tile.TileContext(nc) as tc, ExitStack() as ctx:
        const_pool = ctx.enter_context(tc.tile_pool(name="const", bufs=1))
        dpool = ctx.enter_context(tc.tile_pool(name="dt", bufs=2))
        tpool = ctx.enter_context(tc.tile_pool(name="psum_t", bufs=2, space="PSUM"))
        opool1 = ctx.enter_context(tc.tile_pool(name="psum_o1", bufs=2, space="PSUM"))
        opool2 = ctx.enter_context(tc.tile_pool(name="psum_o2", bufs=2, space="PSUM"))
        apool = ctx.enter_context(tc.tile_pool(name="abs", bufs=3))
        wpool = ctx.enter_context(tc.tile_pool(name="wts", bufs=3))
        spool1 = ctx.enter_context(tc.tile_pool(name="osb1", bufs=2))
        spool2 = ctx.enter_context(tc.tile_pool(name="osb2", bufs=2))

        NF = (YPC // 4) * SW
        xn3_sb = const_pool.tile([128, NF], F16)
        for a in range(4):
            nc.sync.dma_start(xn3_sb[32 * a : 32 * a + 3, :], xn3[3 * a : 3 * a + 3, :])
        c4_sb = const_pool.tile([128, 128], F16)
        nc.sync.dma_start(c4_sb[:], c4[:])

        osb1 = osb2 = None
        for g in range(NG):
            dtk = dpool.tile([128, KBLK * YG * NPLANE], F16, tag="dt")
            nc.sync.dma_start(
                dtk[:].rearrange("p (k c) -> p k c", k=KBLK),
                imgT[g].rearrange("(k p) c -> p k c", k=KBLK),
            )
            for yy in range(YG):
                y = g * YG + yy
                a32, f = 32 * (y % 4), y // 4
                q = yy % 4
                h = yy % 2
                # --- t = xn - p - 128k over stacked windows ---
                psum_t = tpool.tile([128, SW], F32, tag="t")
                lt = c4_sb[a32 : a32 + 3, :]
                nc.tensor.matmul(
                    psum_t[:, 0:512],
                    lhsT=lt,
                    rhs=xn3_sb[a32 : a32 + 3, f * SW : f * SW + 512],
                    start=True,
                    stop=True,
                    tile_position=(a32, 0),
                )
                nc.tensor.matmul(
                    psum_t[:, 512:SW],
                    lhsT=lt,
                    rhs=xn3_sb[a32 : a32 + 3, f * SW + 512 : (f + 1) * SW],
                    start=True,
                    stop=True,
                    tile_position=(a32, 0),
                )
                # --- a = |t| (ACT), wneg = min(a-1, 0) (DVE 4x) ---
                at = apool.tile([128, SW], F16, tag="a")
                nc.scalar.activation(
                    at[:], psum_t[:], mybir.ActivationFunctionType.Abs
                )
                wts = wpool.tile([128, SW], F16, tag="w")
                nc.vector.tensor_scalar(
                    wts[:],
                    at[:],
                    1.0,
                    0.0,
                    op0=mybir.AluOpType.subtract,
                    op1=mybir.AluOpType.min,
                )
                # --- banded matmuls, split-start windows ---
                if q == 0:
                    osb1 = spool1.tile([128, 4 * W], F16, tag="o1")
                if q == 0:
                    osb2 = spool2.tile([128, 2 * W], F16, tag="o2")
                psum_o1 = opool1.tile([128, W], F32, tag="po1")
                if h == 0:
                    psum_o2 = opool2.tile([128, W], F32, tag="po2")
                for k in range(KBLK):
                    base = k * YG * NPLANE + yy * NPLANE
                    l1 = dtk[:, base : base + 128]
                    l2 = dtk[:, base + 128 : base + NPLANE]
                    for (c0, c1, o0, st) in PIECES[k]:
                        nc.tensor.matmul(
                            psum_o1[:, o0 : o0 + (c1 - c0)],
                            lhsT=l1,
                            rhs=wts[:, GL * k + c0 : GL * k + c1],
                            start=st,
                            stop=(k == KBLK - 1 and c1 == GL),
                            skip_group_check=True,
                        )
                    for (c0, c1, o0, st) in PIECES[k]:
                        nc.tensor.matmul(
                            psum_o2[64 * h : 64 * h + 64, o0 : o0 + (c1 - c0)],
                            lhsT=l2,
                            rhs=wts[:, GL * k + c0 : GL * k + c1],
                            start=st,
                            stop=(k == KBLK - 1 and c1 == GL),
                            skip_group_check=True,
                            tile_position=(0, 64 * h),
                        )
                # --- PSUM -> SBUF (negate + fp16): o1 on DVE, o2 on ACT 2x ---
                nc.vector.tensor_scalar_mul(osb1[:, q * W : (q + 1) * W], psum_o1[:], -1.0)
                if h == 1:
                    nc.scalar.mul(
                        osb2[:, (q // 2) * W : (q // 2 + 1) * W], psum_o2[:], -1.0
                    )
                if q == 3:
                    y0 = y - 3
                    pr0 = y0 // 2
                    nc.sync.dma_start(
                        out[0:128, y0 : y0 + 4, :],
                        osb1[:].rearrange("p (y x) -> p y x", y=4),
                    )
                    nc.sync.dma_start(
                        out2[:, :, pr0 : pr0 + 2, :].rearrange(
                            "j p r x -> (j p) (r x)"
                        ),
                        osb2[:],
                    )

    nc.compile()
    return nc


# ---------------- host-side helpers ----------------

def host_xn(c_u: np.ndarray) -> np.ndarray:
    """float64 reproduction of the reference displacement; returns xn [H, W] f32."""
    import math

    CUT = 16
    k = np.arange(1, CUT + 1, dtype=np.float64)
    i, j = np.meshgrid(k, k, indexing="ij")
    r = np.sqrt(i * i + j * j)
    e = (r < CUT + 0.5).astype(np.float64) / r
    x = np.linspace(0.0, 1.0, W, dtype=np.float64)
    s = np.sin(np.pi * x[:, None] * k[None, :])
    u = np.einsum("ij,xi,yj->yx", c_u.astype(np.float64) * e, s, s)
    Tw = 4.0 / (math.pi**3 * CUT**2 * math.log(CUT))
    dx = math.sqrt(Tw) * u * W
    xg = np.arange(W, dtype=np.float64)
    return np.clip(xg[None, :] - dx, 0.0, W - 1.0).astype(np.float32)


def _mask11(v: np.ndarray) -> np.ndarray:
    """Truncate fp32 mantissa to 10 explicit bits (fits fp16 significand)."""
    return (v.view(np.uint32) & np.uint32(0xFFFFE000)).view(np.float32)


def host_prep(img: np.ndarray, c_u: np.ndarray):
    """Build per-core input maps."""
    xn = host_xn(c_u)
    # band-coverage check: every tap P of column x must lie in window P//128
    m = np.floor(xn).astype(np.int64)
    xg = np.arange(W)[None, :]
    for tap in (m, np.minimum(m + 1, W - 1)):
        kk = tap // 128
        rel = xg - np.take(np.asarray(GS + [0]), kk)
        assert (rel >= 0).all() and (rel < GL).all(), "window coverage violated"
    planes = img.reshape(NPLANE, H, W)
    imgT_all = np.ascontiguousarray(
        planes.reshape(NPLANE, NCORES, NG, YG, W)
        .transpose(1, 2, 4, 3, 0)
        .astype(np.float16)
    )
    c4m = np.zeros((128, 128), np.float16)
    p = np.arange(128, dtype=np.float32)
    for a in range(4):
        c4m[32 * a + 0, :] = 1.0
        c4m[32 * a + 1, :] = 1.0
        c4m[32 * a + 2, :] = (-p).astype(np.float16)
    in_maps = []
    NF = (YPC // 4) * SW
    for core in range(NCORES):
        xns = np.zeros((12, NF), np.float16)
        for y in range(YPC):
            a, f = y % 4, y // 4
            yg = core * YPC + y
            for k in range(KBLK):
                seg = xn[yg, GS[k] : GS[k] + GL]
                xh = _mask11(seg)
                xm = (seg - xh).astype(np.float16)
                c0 = f * SW + GL * k
                xns[3 * a + 0, c0 : c0 + GL] = (xh - 128.0 * k).astype(np.float16)
                xns[3 * a + 1, c0 : c0 + GL] = xm
                xns[3 * a + 2, c0 : c0 + GL] = 1.0
        in_maps.append(
            {
                "imgT": imgT_all[core].reshape(NG, W, YG * NPLANE),
                "xn3": xns,
                "c4": c4m,
            }
        )
    return in_maps


def host_gather(outs: list) -> np.ndarray:
    """Assemble per-core 'out'/'out2' into [64, 3, H, W]."""
    full = np.empty((64, 3, H, W), np.float32)
    for core, om in enumerate(outs):
        o1 = om["out"].astype(np.float32)  # [128, YPC, W]
        # out2 [j, pl, pair, x] -> [pl, y=(pair*2+j), x]
        o2 = (
            om["out2"]
            .astype(np.float32)
            .transpose(1, 2, 0, 3)
            .reshape(64, YPC, W)
        )
        o = np.concatenate([o1, o2], axis=0)  # [NPLANE, YPC, W]
        full[:, :, core * YPC : (core + 1) * YPC, :] = o.reshape(64, 3, YPC, W)
    return full


# ---------------- harness entry point ----------------

_NC_CACHE = {}


def kernel(img: "np.ndarray", c_u: "np.ndarray", c_v: "np.ndarray") -> "np.ndarray":
    """Full-input entry: shard across 8 NeuronCores, run, reassemble."""
    img = np.ascontiguousarray(np.asarray(img, dtype=np.float32))
    c_u = np.asarray(c_u, dtype=np.float32)
    in_maps = host_prep(img, c_u)
    if "nc" not in _NC_CACHE:
        _NC_CACHE["nc"] = build_program(num_devices=NCORES)
    from concourse.bass_utils import run_bass_kernel_spmd

    res = run_bass_kernel_spmd(
        _NC_CACHE["nc"], in_maps, core_ids=list(range(NCORES)), trace=False
    )
    return host_gather(res.results)
